# revision 18
# baseline (speedup 1.0000x reference)
"""Trainium2 Bass kernel for nn_MetaController (GRU meta-controller).

Architecture (B=4, N=512, D=512, H=1024, R=16):
  - 2 GRUs (action-proposer, switching-unit) over N=512 sequential steps
  - reparameterized sampling, sigmoid beta gate
  - gated linear scan over time (tensor_tensor_scan)
  - decoder MLP -> low-rank hypernetwork; algebraic simplifications:
      * w2-half of dec_w2 only appears as sum over d -> pre-reduced on host
      * y[d] = sum_r w1[d,r] * s2[r] via r-major GEMM + DVE contraction

Sharding (8 cores, identical SPMD program, per-core *data* differs):
  core c: batch b = c % 4, rank-half role = c // 4.
  - each core runs BOTH GRU chains (ap + su) for its batch, interleaved:
    the recurrence is latency-bound (PE burst 1.3us + serial gate tail
    1.7us per step), so chain B's matmul burst hides under chain A's tail
  - GRU outputs are local -> no h exchange collective at all
  - decoder W2a GEMM sharded by rank-half; partial y summed with an
    AllReduce over {c, c+4}; host reads cores 0-3

Precision: W_hh + moving h in bf16 (fp32 recurrence state kept in SBUF),
all large GEMMs in float32r (TF32-like, 1 cyc/row), everything else fp32.
"""

import sys

sys.path.insert(0, "/opt/trn_rl_repo")

import numpy as np

import concourse.bass as bass
import concourse.tile as tile
from concourse import bacc, mybir
from concourse.bass_utils import run_bass_kernel_spmd

F32 = mybir.dt.float32
F32R = mybir.dt.float32r
BF16 = mybir.dt.bfloat16
AF = mybir.ActivationFunctionType
ALU = mybir.AluOpType

B, N, D = 4, 512, 512
G = 3 * D            # 1536 gate width
H = 1024             # decoder hidden
R = 16               # low rank
P = 128
DC = D // P          # 4 d-chunks
GC = G // P          # 12 gate chunks
HC = H // P          # 8 hidden chunks
RH = R // 2          # 8 ranks per core
NCORES = 8
QUADS = [[i, i + 4] for i in range(4)]   # y partial-sum pairs

# precision knobs
GRU_MODE = "bf16"    # "f32" | "bf16" | "split" (hi/lo bf16, ~fp32 accuracy)
GRU_DT = F32 if GRU_MODE == "f32" else BF16
XP_DT = F32 if GRU_MODE in ("f32", "split") else BF16


def _build_program(nsteps=N):
    nc = bacc.Bacc("TRN2", target_bir_lowering=False, debug=False,
                   num_devices=NCORES)

    def din(name, shape, dt=F32):
        return nc.dram_tensor(name, list(shape), dt, kind="ExternalInput").ap()

    xT_d = din("xT", [D, N], F32R)              # residual[b].T
    noiseT_d = din("noiseT", [D, N])
    wih_d = [din(f"wihT{s}", [D, G], F32R) for s in "AB"]
    whh_d = [din(f"whhT{s}", [D, G], GRU_DT) for s in "AB"]
    if GRU_MODE == "split":
        whhlo_d = [din(f"whhLo{s}", [D, G], BF16) for s in "AB"]
    xbias_d = [din(f"xbias{s}", [P, GC]) for s in "AB"]
    bhhn_d = [din(f"bhhn{s}", [P, DC], XP_DT) for s in "AB"]
    aowT_d = din("aowT", [D, 2 * D], F32R)      # ap_out_w^T
    bwrep_d = din("bwrep", [D, P], F32R)        # beta_w^T replicated to 128 cols
    dw1T_d = din("dw1T", [D, H], F32R)          # dec_w1^T
    db1_d = din("db1", [P, HC])
    w2a_d = din("w2a", [RH * DC, P, HC, P], F32R)  # pre-tiled lhsT chunks
    b2a_d = din("b2a", [P, RH * DC])
    w2sT_d = din("w2sT", [H, R], F32R)          # pre-reduced w2-half
    b2s_d = din("b2s", [R, 1])
    sel_d = din("sel", [R, RH * P], F32R)       # row-selectors for r bcast
    ident_d = din("ident", [P, P], XP_DT)       # identity for psum preloads

    outT_d = nc.dram_tensor("outT", [P, DC, N], F32, kind="ExternalOutput").ap()

    with tile.TileContext(nc) as tc:
        from contextlib import ExitStack
        with ExitStack() as ctx:
            perm = ctx.enter_context(tc.tile_pool(name="perm", bufs=1))
            ppb = ctx.enter_context(tc.tile_pool(name="ppb", bufs=2, space="PSUM"))
            pps = ctx.enter_context(tc.tile_pool(name="pps", bufs=2, space="PSUM"))
            dram = ctx.enter_context(tc.tile_pool(name="dram", bufs=1, space="DRAM"))

            xT_sb = perm.tile([P, DC, N], F32R)
            nc.sync.dma_start(xT_sb[:], xT_d.rearrange("(k p) t -> p k t", p=P))
            h_sb = [perm.tile([P, DC, N], F32R, name=f"h{s}") for s in "AB"]
            gated_sb = perm.tile([P, DC, N], F32R)

            # ------------- phase 1+2: xp GEMMs, dual GRU recurrence -------------
            with tc.tile_pool(name="gru", bufs=1) as pg:
                whh_sb, whhlo_sb, xp_sb, bhhn_sb, h16, hlo16 = \
                    [], [], [], [], [], []
                xpn_sb = []
                for ci, s in enumerate("AB"):
                    w = pg.tile([P, DC, G], GRU_DT, name=f"whh{s}")
                    nc.sync.dma_start(w[:],
                                      whh_d[ci].rearrange("(k p) g -> p k g", p=P))
                    whh_sb.append(w)
                    if GRU_MODE == "split":
                        wl = pg.tile([P, DC, G], BF16, name=f"whhlo{s}")
                        nc.sync.dma_start(
                            wl[:], whhlo_d[ci].rearrange("(k p) g -> p k g", p=P))
                        whhlo_sb.append(wl)
                    xp_sb.append(pg.tile([P, 8, N], XP_DT, name=f"xp{s}"))
                    xpn_sb.append(pg.tile([P, DC, N], F32, name=f"xpn{s}"))
                    bh = pg.tile([P, DC], XP_DT, name=f"bhhn{s}")
                    nc.sync.dma_start(bh[:], bhhn_d[ci][:])
                    bhhn_sb.append(bh)
                    if GRU_MODE != "f32":
                        h16.append(pg.tile([P, DC, N], BF16, name=f"h16{s}"))
                    if GRU_MODE == "split":
                        hlo16.append(pg.tile([P, DC, N], BF16, name=f"hlo{s}"))
                ident_sb = pg.tile([P, P], XP_DT)
                nc.sync.dma_start(ident_sb[:], ident_d[:])

                # xp = x @ W_ih^T + folded biases (per chain)
                with tc.tile_pool(name="ph1", bufs=4) as p1:
                    for ci in range(2):
                        xb = p1.tile([P, GC], F32, name="xb", bufs=2)
                        nc.sync.dma_start(xb[:], xbias_d[ci][:])
                        for m in range(GC):
                            ps = ppb.tile([P, N], F32, name="ps_xp", tag="psbig")
                            for k in range(DC):
                                wc = p1.tile([P, P], F32R, name="wih_c")
                                nc.sync.dma_start(
                                    wc[:], wih_d[ci][k * P:(k + 1) * P,
                                                     m * P:(m + 1) * P])
                                nc.tensor.matmul(ps[:], lhsT=wc[:],
                                                 rhs=xT_sb[:, k, :],
                                                 start=(k == 0), stop=(k == DC - 1))
                            dst = (xp_sb[ci][:, m, :] if m < 8
                                   else xpn_sb[ci][:, m - 8, :])
                            nc.scalar.activation(dst, ps[:],
                                                 AF.Identity, bias=xb[:, m:m + 1])

                def h_mm_rhs(ci, k, t):
                    if GRU_MODE == "f32":
                        return h_sb[ci][:, k, t:t + 1]
                    return h16[ci][:, k, t:t + 1]

                def emit_gate_mms(ci, ps_col, j, t):
                    gsl = slice(j * P, (j + 1) * P)
                    pairs = []
                    for k in range(DC):
                        pairs.append((whh_sb[ci][:, k, gsl], h_mm_rhs(ci, k, t - 1)))
                        if GRU_MODE == "split":
                            pairs.append((whh_sb[ci][:, k, gsl],
                                          hlo16[ci][:, k, t - 1:t]))
                            pairs.append((whhlo_sb[ci][:, k, gsl],
                                          h16[ci][:, k, t - 1:t]))
                    for i, (lw, rh) in enumerate(pairs):
                        nc.tensor.matmul(ps_col, lhsT=lw, rhs=rh,
                                         start=False, stop=(i == len(pairs) - 1),
                                         skip_group_check=True)

                def emit_h_out(ci, nn, tz, t):
                    """h = n + tz (tz = z*(h_prev-n)); bf16 copy first."""
                    if GRU_MODE == "f32":
                        nc.vector.tensor_tensor(h_sb[ci][:, :, t], nn[:], tz[:],
                                                ALU.add)
                        return
                    nc.vector.tensor_tensor(h16[ci][:, :, t], nn[:], tz[:], ALU.add)
                    nc.vector.tensor_tensor(h_sb[ci][:, :, t], nn[:], tz[:], ALU.add)
                    if GRU_MODE == "split":
                        lo = pg.tile([P, DC], F32, name="hlo_s", bufs=2)
                        nc.vector.tensor_tensor(lo[:], h_sb[ci][:, :, t].bitcast(F32),
                                                h16[ci][:, :, t], ALU.subtract)
                        nc.scalar.activation(hlo16[ci][:, :, t], lo[:], AF.Copy)

                def emit_step0(ci):
                    rz0 = pg.tile([P, 8], F32, name="rz_s", bufs=2)
                    nc.scalar.activation(rz0[:], xp_sb[ci][:, 0:8, 0], AF.Sigmoid)
                    t1 = pg.tile([P, DC], F32, name="t1_s", bufs=2)
                    nc.vector.tensor_tensor(t1[:], rz0[:, 0:4], bhhn_sb[ci][:],
                                            ALU.mult)
                    nc.vector.tensor_tensor(t1[:], t1[:], xpn_sb[ci][:, :, 0],
                                            ALU.add)
                    n0 = pg.tile([P, DC], F32, name="n_s", bufs=2)
                    nc.scalar.activation(n0[:], t1[:], AF.Tanh)
                    tz = pg.tile([P, DC], F32, name="tz_s", bufs=2)
                    nc.vector.tensor_tensor(tz[:], rz0[:, 4:8], n0[:], ALU.mult)
                    nc.vector.tensor_tensor(tz[:], tz[:], tz[:], ALU.subtract)
                    # h0 = (1-z)*n = n - z*n -> reuse emit path: tz2 = -z*n
                    tz2 = pg.tile([P, DC], F32, name="tz2_s", bufs=2)
                    nc.vector.tensor_tensor(tz2[:], rz0[:, 4:8], n0[:], ALU.mult)
                    nc.vector.tensor_tensor(tz2[:], tz[:], tz2[:], ALU.subtract)
                    emit_h_out(ci, n0, tz2, 0)

                def emit_step(ci, t):
                    psB = pps.tile([P, DC], F32, name="psB")
                    psAr = pps.tile([P, DC], F32, name="psAr")
                    psAz = pps.tile([P, DC], F32, name="psAz")
                    # psum preloads (independent of h[t-1]; run under the
                    # previous tail). One wide start=True matmul per bank.
                    nc.tensor.matmul(psB[:, :], lhsT=ident_sb[:],
                                     rhs=bhhn_sb[ci][:, :], start=True, stop=False)
                    nc.tensor.matmul(psAr[:, :], lhsT=ident_sb[:],
                                     rhs=xp_sb[ci][:, 0:4, t], start=True,
                                     stop=False)
                    nc.tensor.matmul(psAz[:, :], lhsT=ident_sb[:],
                                     rhs=xp_sb[ci][:, 4:8, t], start=True,
                                     stop=False)
                    for j in range(8, GC):
                        emit_gate_mms(ci, psB[:, j - 8:j - 7], j, t)
                    for j in range(0, 4):
                        emit_gate_mms(ci, psAr[:, j:j + 1], j, t)
                    for j in range(4, 8):
                        emit_gate_mms(ci, psAz[:, j - 4:j - 3], j, t)
                    rr_ = pg.tile([P, DC], F32, name="r_s", bufs=2)
                    nc.scalar.activation(rr_[:], psAr[:], AF.Sigmoid)
                    zz_ = pg.tile([P, DC], F32, name="z_s", bufs=2)
                    nc.scalar.activation(zz_[:], psAz[:], AF.Sigmoid)
                    t1 = pg.tile([P, DC], F32, name="t1_s", bufs=2)
                    nc.vector.tensor_tensor(t1[:], rr_[:], psB[:], ALU.mult)
                    nc.vector.tensor_tensor(t1[:], t1[:], xpn_sb[ci][:, :, t],
                                            ALU.add)
                    nn = pg.tile([P, DC], F32, name="n_s", bufs=2)
                    nc.scalar.activation(nn[:], t1[:], AF.Tanh)
                    tz = pg.tile([P, DC], F32, name="tz_s", bufs=2)
                    nc.vector.tensor_tensor(tz[:], h_sb[ci][:, :, t - 1].bitcast(F32),
                                            nn[:], ALU.subtract)
                    nc.vector.tensor_tensor(tz[:], zz_[:], tz[:], ALU.mult)
                    emit_h_out(ci, nn, tz, t)

                emit_step0(0)
                emit_step0(1)
                for t in range(1, nsteps):
                    emit_step(0, t)
                    emit_step(1, t)

            # ------------- phase 4: sampled, beta, gated scan (local h) ---------
            with tc.tile_pool(name="ph4", bufs=1) as p4, \
                 tc.tile_pool(name="wch", bufs=4) as wch:
                noise_sb = p4.tile([P, DC, N], F32)
                nc.sync.dma_start(noise_sb[:],
                                  noiseT_d.rearrange("(k p) t -> p k t", p=P))
                bwrep_sb = p4.tile([P, DC, P], F32R)
                nc.sync.dma_start(bwrep_sb[:],
                                  bwrep_d.rearrange("(k p) m -> p k m", p=P))

                beta_sb = p4.tile([P, N], F32)
                psb = ppb.tile([P, N], F32, name="ps_beta", tag="psbig")
                for k in range(DC):
                    nc.tensor.matmul(psb[:], lhsT=bwrep_sb[:, k, :],
                                     rhs=h_sb[1][:, k, :],
                                     start=(k == 0), stop=(k == DC - 1))
                nc.scalar.activation(beta_sb[:], psb[:], AF.Sigmoid)
                forget_sb = p4.tile([P, N], F32)
                nc.scalar.activation(forget_sb[:], beta_sb[:], AF.Identity,
                                     bias=1.0, scale=-1.0)

                samp_sb = p4.tile([P, DC, N], F32)
                for k in range(DC):
                    psm = ppb.tile([P, N], F32, name="ps_mean", tag="psbig")
                    psv = ppb.tile([P, N], F32, name="ps_lv", tag="psbig")
                    for kk in range(DC):
                        mch = wch.tile([P, P], F32R, name="aow_m")
                        nc.sync.dma_start(
                            mch[:], aowT_d[kk * P:(kk + 1) * P, k * P:(k + 1) * P])
                        nc.tensor.matmul(psm[:], lhsT=mch[:],
                                         rhs=h_sb[0][:, kk, :],
                                         start=(kk == 0), stop=(kk == DC - 1))
                    for kk in range(DC):
                        vch = wch.tile([P, P], F32R, name="aow_v")
                        nc.sync.dma_start(
                            vch[:], aowT_d[kk * P:(kk + 1) * P,
                                           (DC + k) * P:(DC + k + 1) * P])
                        nc.tensor.matmul(psv[:], lhsT=vch[:],
                                         rhs=h_sb[0][:, kk, :],
                                         start=(kk == 0), stop=(kk == DC - 1))
                    std = p4.tile([P, N], F32, name="std_t", bufs=2)
                    nc.scalar.activation(std[:], psv[:], AF.Exp, scale=0.5)
                    nc.vector.tensor_tensor(std[:], noise_sb[:, k, :], std[:],
                                            ALU.mult)
                    nc.vector.tensor_tensor(samp_sb[:, k, :], psm[:], std[:],
                                            ALU.add)
                    nc.vector.tensor_tensor(samp_sb[:, k, :], samp_sb[:, k, :],
                                            forget_sb[:], ALU.mult)
                    nc.vector.tensor_tensor_scan(gated_sb[:, k, :], beta_sb[:],
                                                 samp_sb[:, k, :], 0.0,
                                                 ALU.mult, ALU.add)

            # ------------- phase 5: decoder (rank-half shard) -------------------
            y_dr = dram.tile([P, DC, N], F32)
            ysum_dr = dram.tile([P, DC, N], F32)
            with tc.tile_pool(name="ph5", bufs=1) as p5, \
                 tc.tile_pool(name="w2p", bufs=3) as w2p, \
                 tc.tile_pool(name="s2p", bufs=2) as s2p:
                db1_sb = p5.tile([P, HC], F32)
                nc.sync.dma_start(db1_sb[:], db1_d[:])
                hid_sb = p5.tile([P, HC, N], F32R)
                with tc.tile_pool(name="dw1", bufs=4) as dw1p:
                    for m in range(HC):
                        ps = ppb.tile([P, N], F32, name="ps_hid", tag="psbig")
                        for k in range(DC):
                            wc = dw1p.tile([P, P], F32R, name="dw1_c")
                            nc.sync.dma_start(
                                wc[:], dw1T_d[k * P:(k + 1) * P, m * P:(m + 1) * P])
                            nc.tensor.matmul(ps[:], lhsT=wc[:],
                                             rhs=gated_sb[:, k, :],
                                             start=(k == 0), stop=(k == DC - 1))
                        nc.scalar.activation(hid_sb[:, m, :], ps[:], AF.Silu,
                                             bias=db1_sb[:, m:m + 1])

                w2sT_sb = p5.tile([P, HC, R], F32R)
                nc.sync.dma_start(w2sT_sb[:],
                                  w2sT_d.rearrange("(k p) r -> p k r", p=P))
                b2s_sb = p5.tile([R, 1], F32)
                nc.sync.dma_start(b2s_sb[:], b2s_d[:])
                s2big = p5.tile([R, N], F32R)
                ps2 = ppb.tile([R, N], F32, name="ps_s2", tag="psbig")
                for kk in range(HC):
                    nc.tensor.matmul(ps2[:], lhsT=w2sT_sb[:, kk, :],
                                     rhs=hid_sb[:, kk, :],
                                     start=(kk == 0), stop=(kk == HC - 1))
                nc.scalar.activation(s2big[:], ps2[:], AF.Identity,
                                     bias=b2s_sb[:])

                sel_sb = p5.tile([R, RH * P], F32R)
                nc.sync.dma_start(sel_sb[:], sel_d[:])
                b2a_sb = p5.tile([P, RH * DC], F32)
                nc.sync.dma_start(b2a_sb[:], b2a_d[:])

                y_sb = p5.tile([P, DC, N], F32)
                for rl in range(RH):
                    pbc = ppb.tile([P, N], F32, name="ps_bc", tag="psbig")
                    nc.tensor.matmul(pbc[:], lhsT=sel_sb[:, rl * P:(rl + 1) * P],
                                     rhs=s2big[:], start=True, stop=True)
                    s2bc = s2p.tile([P, N], F32, name="s2bc")
                    nc.vector.tensor_copy(out=s2bc[:], in_=pbc[:])
                    for db in range(DC):
                        cidx = rl * DC + db
                        w2c = w2p.tile([P, HC, P], F32R, name="w2c")
                        nc.sync.dma_start(w2c[:], w2a_d[cidx])
                        pw = ppb.tile([P, N], F32, name="ps_w1", tag="psbig")
                        for kk in range(HC):
                            nc.tensor.matmul(pw[:], lhsT=w2c[:, kk, :],
                                             rhs=hid_sb[:, kk, :],
                                             start=(kk == 0), stop=(kk == HC - 1))
                        if rl == 0:
                            nc.vector.scalar_tensor_tensor(
                                y_sb[:, db, :], pw[:], b2a_sb[:, cidx:cidx + 1],
                                s2bc[:], ALU.add, ALU.mult)
                        else:
                            tmp = s2p.tile([P, N], F32, name="ytmp")
                            nc.vector.scalar_tensor_tensor(
                                tmp[:], pw[:], b2a_sb[:, cidx:cidx + 1],
                                s2bc[:], ALU.add, ALU.mult)
                            nc.vector.tensor_tensor(y_sb[:, db, :], y_sb[:, db, :],
                                                    tmp[:], ALU.add)

                # partial-y sum across the rank-half pair {c, c+4}
                nc.sync.dma_start(y_dr[:], y_sb[:])
                nc.gpsimd.collective_compute(
                    "AllReduce", ALU.add, replica_groups=QUADS,
                    ins=[y_dr.opt()], outs=[ysum_dr.opt()])
                ysum_sb = p5.tile([P, DC, N], F32)
                nc.sync.dma_start(ysum_sb[:], ysum_dr[:])

                out_sb = p5.tile([P, DC, N], F32)
                for k in range(DC):
                    nc.vector.tensor_tensor(out_sb[:, k, :],
                                            gated_sb[:, k, :].bitcast(F32),
                                            ysum_sb[:, k, :], ALU.mult)
                    nc.vector.tensor_tensor(out_sb[:, k, :], out_sb[:, k, :],
                                            xT_sb[:, k, :].bitcast(F32), ALU.add)
                nc.sync.dma_start(outT_d[:], out_sb[:])

    nc.compile()
    return nc


_PROG = {}


def _get_program(nsteps=N):
    if nsteps not in _PROG:
        _PROG[nsteps] = _build_program(nsteps)
    return _PROG[nsteps]


def _prep_in_maps(inputs):
    f = np.float32
    bf = np.dtype("bfloat16")
    np_xp = f if XP_DT == F32 else bf
    np_gru = f if GRU_MODE == "f32" else bf
    res = np.asarray(inputs["residual_stream"], f)
    noi = np.asarray(inputs["noise"], f)
    gru_w = [
        (np.asarray(inputs["ap_w_ih"], f), np.asarray(inputs["ap_w_hh"], f),
         np.asarray(inputs["ap_b_ih"], f), np.asarray(inputs["ap_b_hh"], f)),
        (np.asarray(inputs["su_w_ih"], f), np.asarray(inputs["su_w_hh"], f),
         np.asarray(inputs["su_b_ih"], f), np.asarray(inputs["su_b_hh"], f)),
    ]
    aowT = np.ascontiguousarray(np.asarray(inputs["ap_out_w"], f).T)      # [D, 2D]
    bwrep = np.ascontiguousarray(
        np.tile(np.asarray(inputs["beta_w"], f).reshape(D, 1), (1, P)))   # [D, P]
    dw1T = np.ascontiguousarray(np.asarray(inputs["dec_w1"], f).T)        # [D, H]
    db1 = np.ascontiguousarray(
        np.asarray(inputs["dec_b1"], f).reshape(HC, P).T)                 # [P, HC]
    w2 = np.asarray(inputs["dec_w2"], f)                                  # [2DR, H]
    b2 = np.asarray(inputs["dec_b2"], f)                                  # [2DR]
    W2a = w2[:D * R].reshape(D, R, H)                                     # [d, r, h]
    B2a = b2[:D * R].reshape(D, R)
    W2s = w2[D * R:].reshape(D, R, H).sum(axis=0)                         # [R, H]
    b2s = b2[D * R:].reshape(D, R).sum(axis=0).reshape(R, 1)              # [R, 1]
    w2sT = np.ascontiguousarray(W2s.T)                                    # [H, R]
    ident = np.eye(P, dtype=f).astype(np_xp)

    # per-chain GRU params (shared by all cores)
    chain = []
    for w_ih, w_hh, b_ih, b_hh in gru_w:
        whhT = np.ascontiguousarray(w_hh.T)
        xbias = b_ih + np.concatenate([b_hh[:2 * D], np.zeros(D, f)])
        d = {
            "wihT": np.ascontiguousarray(w_ih.T),
            "whhT": whhT.astype(np_gru),
            "xbias": np.ascontiguousarray(xbias.reshape(GC, P).T),
            "bhhn": np.ascontiguousarray(
                b_hh[2 * D:].reshape(DC, P).T).astype(np_xp),
        }
        if GRU_MODE == "split":
            hi = whhT.astype(bf)
            d["whhLo"] = (whhT - hi.astype(f)).astype(bf)
        chain.append(d)

    in_maps = []
    for c in range(NCORES):
        b, role = c % 4, c // 4
        rsl = slice(role * RH, (role + 1) * RH)
        sub = W2a[:, rsl, :]                                              # [D, RH, H]
        t = sub.transpose(1, 0, 2).reshape(RH, DC, P, H)                  # [rl,db,m,h]
        w2a_tiled = np.ascontiguousarray(
            t.transpose(0, 1, 3, 2).reshape(RH * DC, HC, P, P)
            .transpose(0, 2, 1, 3))                                       # [cidx,p,kk,m]
        b2a_c = np.zeros((P, RH * DC), f)
        for rl in range(RH):
            for db in range(DC):
                b2a_c[:, rl * DC + db] = B2a[db * P:(db + 1) * P, role * RH + rl]
        sel = np.zeros((R, RH * P), f)
        for rl in range(RH):
            sel[role * RH + rl, rl * P:(rl + 1) * P] = 1.0

        im = {
            "xT": np.ascontiguousarray(res[b].T),
            "noiseT": np.ascontiguousarray(noi[b].T),
            "aowT": aowT, "bwrep": bwrep, "dw1T": dw1T, "db1": db1,
            "w2a": w2a_tiled, "b2a": b2a_c, "w2sT": w2sT, "b2s": b2s,
            "sel": sel, "ident": ident,
        }
        for ci, s in enumerate("AB"):
            for k, v in chain[ci].items():
                im[k + s if k in ("wihT", "whhT", "xbias", "bhhn", "whhLo")
                   else k] = v
        in_maps.append(im)
    return in_maps


def kernel(**inputs):
    nc = _get_program()
    in_maps = _prep_in_maps(inputs)
    rr = run_bass_kernel_spmd(nc, in_maps, list(range(NCORES)))
    modified = np.empty((B, N, D), np.float32)
    for b in range(B):
        o = rr.results[b]["outT"]                          # [P, DC, N]
        modified[b] = o.transpose(2, 1, 0).reshape(N, D)
    return modified, np.zeros((), np.float32)


def _install_ntff_shim():
    import types
    if "antenv.axon_hooks" in sys.modules:
        return
    mod = types.ModuleType("antenv.axon_hooks")
    holder = {}
    mod.set_axon_ntff_profile_hook = lambda h: holder.__setitem__("h", h)
    mod.get_axon_ntff_profile_hook = lambda: holder.get("h")
    sys.modules["antenv.axon_hooks"] = mod
    import antenv
    antenv.axon_hooks = mod
    from trn_agent_boot.trn_boot import _ntff_profile_via_ctypes
    mod.set_axon_ntff_profile_hook(
        _ntff_profile_via_ctypes("/opt/axon/libaxon_pjrt.so"))


def profile_once(inputs, trace_kwargs=None, tmpdir=None):
    import tempfile
    import concourse.bass_utils as bu
    _install_ntff_shim()
    bu.upload_artifacts = lambda d: str(d)
    nc = _get_program()
    in_maps = _prep_in_maps(inputs)
    if tmpdir is None:
        tmpdir = tempfile.mkdtemp(prefix="ntff_")
    rr = run_bass_kernel_spmd(nc, in_maps, list(range(NCORES)), trace=True,
                              tmpdir=tmpdir, trace_kwargs=trace_kwargs or {})
    return rr


if __name__ == "__main__":
    import time
    nsteps = int(sys.argv[1]) if len(sys.argv) > 1 else N
    t0 = time.time()
    nc = _build_program(nsteps)
    print(f"build+compile nsteps={nsteps}:", time.time() - t0)
